# revision 39
# baseline (speedup 1.0000x reference)
"""Trainium2 Bass kernel for AdaptedEnzymeModel (per-node MLP -> segment mean
pool -> graph MLP), SPMD over 8 NeuronCores.

Strategy (hybrid PE pipeline + table gather)
--------------------------------------------
* BN affines folded into adjacent Linears on host; device runs Linear+ReLU
  chains in bf16 (fp32 PSUM).
* Node dim sharded at graph boundaries: core c owns graphs [512c, 512c+512),
  4 groups of 128 graphs, each padded to G nodes (multiple of 2048).
* Because the node MLP has input dimension 1, z6 = MLP(x) is a function of a
  single scalar.  The host tabulates it on a 32768-level grid over x
  (step 1/2048, quantization error ~2e-4) as a [32768, 128] bf16 table.
  The last NTG tiles of each group are produced by SWDGE dma_gather
  (256B rows keyed by the quantized x) on the otherwise-idle GpSimd engine,
  bypassing the PE and both evacuation engines entirely for ~25% of nodes.
* The remaining tiles run the PE pipeline, 2-tile macros with the proven
  deferred-phase overlap:
  - L1 is an outer product: one K=64 selector matmul computes four tiles at
    once into one [128 = 4x32 feats, 512] bank (amortized over 2 macros).
  - L2: block-diagonal K=64 weights at row position 0/64 per macro parity.
  - L3/L4: 2-tile block-diagonal K=128 matmuls (one per macro).
  - L5 (K=64/tile, M=128): row-split pairs at positions (0,0)/(64,0), which
    stream concurrently on disjoint PE row bands.
  - L6 in [node, feat] layout: ones-row bias prefill + 4 K=128 block
    matmuls per tile, deferred one macro so its PE work hides evacuations.
* Segment mean-pool without one-hot streams: every 128-node block spans at
  most 2 graphs (min graph size 192), so a per-tile [128, 8] "split one-hot"
  (2 run columns per block) turns the pooling into N=2 slot matmuls that
  accumulate into a per-group [128, 8*NT] slot PSUM bank; gathered tiles
  feed the same slot matmuls.  Stage 2 (per group): evacuate slots,
  PE-transpose via identity, multiply by a [slots, bins] one-hot to get
  per-graph sums, scale by exact fp32 1/count.
* Evacuations are balanced across Scalar and Vector by phase/parity.
* Final graph MLP (128->64->32->7) on-device.
"""

import numpy as np
import ml_dtypes
from contextlib import ExitStack

import concourse.bass as bass
import concourse.tile as tile
from concourse import bacc, mybir, library_config
from concourse.bass_utils import run_bass_kernel_spmd

NCORES = 8
GROUPS = 4          # bin-groups per core
BINS = 128          # graphs per group
NCLS = 7
EPS = 1e-5
F32 = mybir.dt.float32
BF16 = mybir.dt.bfloat16
NPBF = ml_dtypes.bfloat16
RELU = mybir.ActivationFunctionType.Relu
ALU = mybir.AluOpType

LAST_RESULT = None
_NC_CACHE = {}
I16 = mybir.dt.int16
NTGS = (14, 16, 20, 24)   # gather-sourced tiles per group (later groups
                          # lean on the gather queue that outlives their PE)
MAXGP = max(NTGS) // 2
XLO, XSTEP = -8.0, 1.0 / 2048.0   # x quantization grid for the z6 table
NROWS = 32768


def _ensure_ntff_hook():
    """bass_utils' trace path needs antenv.axon_hooks, which this image's
    antenv package lacks.  Register a shim backed by trn_agent_boot's ctypes
    NTFF driver so BASS_TRACE=1 yields exec_time_ns.  Degrades silently."""
    import sys
    import types
    try:
        import antenv
        if "antenv.axon_hooks" in sys.modules:
            return
        mod = types.ModuleType("antenv.axon_hooks")
        mod._hook = None
        mod.set_axon_ntff_profile_hook = lambda h: setattr(mod, "_hook", h)
        mod.get_axon_ntff_profile_hook = lambda: mod._hook
        sys.modules["antenv.axon_hooks"] = mod
        antenv.axon_hooks = mod
        from trn_agent_boot.trn_boot import _ntff_profile_via_ctypes
        mod._hook = _ntff_profile_via_ctypes("/opt/axon/libaxon_pjrt.so")
    except Exception:
        pass


_ensure_ntff_hook()


# ---------------------------------------------------------------- host math --
def _fold(p):
    """Fold eval-mode BN affines into adjacent linears. Returns dict of f32."""
    def aff(bn):
        g, b, m, v = bn[0], bn[1], bn[2], bn[3]
        s = g / np.sqrt(v + EPS)
        return s.astype(np.float32), (b - m * s).astype(np.float32)

    s1, t1 = aff(p["ne_bn1"]); s2, t2 = aff(p["ne_bn2"])
    sc1, tc1 = aff(p["cbn1"]); sc2, tc2 = aff(p["cbn2"])
    sf1, tf1 = aff(p["fbn1"]); sf2, tf2 = aff(p["fbn2"])
    f = {}
    f["W1"] = p["ne_w1"]; f["B1"] = p["ne_b1"]
    f["W2"] = s1[:, None] * p["ne_w2"]; f["B2"] = t1 @ p["ne_w2"] + p["ne_b2"]
    f["W3"] = s2[:, None] * p["c1a_w"]; f["B3"] = t2 @ p["c1a_w"] + p["c1a_b"]
    f["W4"] = p["c1b_w"];               f["B4"] = p["c1b_b"]
    f["W5"] = sc1[:, None] * p["c2a_w"]; f["B5"] = tc1 @ p["c2a_w"] + p["c2a_b"]
    f["W6"] = p["c2b_w"];               f["B6"] = p["c2b_b"]
    f["F1"] = sc2[:, None] * p["f1_w"]; f["F1B"] = tc2 @ p["f1_w"] + p["f1_b"]
    f["F2"] = sf1[:, None] * p["f2_w"]; f["F2B"] = tf1 @ p["f2_w"] + p["f2_b"]
    f["F3"] = sf2[:, None] * p["f3_w"]; f["F3B"] = tf2 @ p["f3_w"] + p["f3_b"]
    return {k: np.asarray(v, np.float32) for k, v in f.items()}


# bf16 const block layout
def _layout_bf():
    off, c = {}, 0
    for name, ncols in [("W2D", 128),    # blockdiag(W2pad, W2pad) K=64, M=128
                        ("W3D", 128),    # blockdiag(W3, W3) K=128, M=128
                        ("W4D", 128),
                        ("W5R", 128),    # W5 replicated on rows 0:64 / 64:128
                        ("W6", 128),
                        ("ONES", 128),
                        ("B6R4", 512), ("B6BC", 512)]:
        off[name] = c
        c += ncols
    return off, c


# f32 const block: biases + final mlp + identity (for PE transpose)
def _layout_fp():
    off, c = {}, 0
    for name, ncols in [("F1", 64), ("F2", 32), ("F3", NCLS), ("IDN", 128),
                        ("B6P", 512),
                        ("B1Q", 1),   # B1 tiled x4 (4-tile packed z1)
                        ("B2S", 1), ("B3S", 1), ("B4S", 1),
                        ("B5", 1), ("F1B", 1), ("F2B", 1), ("F3B", 1)]:
        off[name] = c
        c += ncols
    return off, c


_OFFB, _CWB = _layout_bf()
_OFFF, _CWF = _layout_fp()


def _pack_consts(f):
    wb = np.zeros((128, _CWB), NPBF)

    def putb(name, arr):
        wb[:arr.shape[0], _OFFB[name]:_OFFB[name] + arr.shape[1]] = \
            arr.astype(NPBF)

    # W2D stacked twice so both (0,0) and (64,0) row positions can read it
    w2d = np.zeros((128, 128), np.float32)
    for h in (0, 64):
        w2d[h + 0:h + 32, 0:64] = f["W2"]
        w2d[h + 32:h + 64, 64:128] = f["W2"]
    putb("W2D", w2d)
    w3d = np.zeros((128, 128), np.float32)
    w3d[0:64, 0:64] = f["W3"]
    w3d[64:128, 64:128] = f["W3"]
    putb("W3D", w3d)
    w4d = np.zeros((128, 128), np.float32)
    w4d[0:64, 0:64] = f["W4"]
    w4d[64:128, 64:128] = f["W4"]
    putb("W4D", w4d)
    putb("W5R", np.tile(f["W5"], (2, 1)))
    putb("W6", f["W6"])
    wb[0, _OFFB["ONES"]:_OFFB["ONES"] + 128] = NPBF(1.0)
    wb[0, _OFFB["B6R4"]:_OFFB["B6R4"] + 512] = np.tile(f["B6"].astype(NPBF), 4)
    wb[:, _OFFB["B6BC"]:_OFFB["B6BC"] + 512] = \
        np.tile(f["B6"], 4)[None, :].astype(NPBF)

    wf = np.zeros((128, _CWF), np.float32)
    for k in ["F1", "F2", "F3"]:
        arr = f[k]
        wf[:arr.shape[0], _OFFF[k]:_OFFF[k] + arr.shape[1]] = arr
    wf[:, _OFFF["IDN"]:_OFFF["IDN"] + 128] = np.eye(128, dtype=np.float32)
    wf[:, _OFFF["B6P"]:_OFFF["B6P"] + 512] = np.tile(f["B6"], 4)[None, :]
    wf[:, _OFFF["B1Q"]] = np.tile(f["B1"], 4)
    wf[:, _OFFF["B2S"]] = np.tile(f["B2"], 2)
    wf[:, _OFFF["B3S"]] = np.tile(f["B3"], 2)
    wf[:, _OFFF["B4S"]] = np.tile(f["B4"], 2)
    wf[:128, _OFFF["B5"]] = f["B5"]
    for k, d in [("F1B", 64), ("F2B", 32), ("F3B", NCLS)]:
        wf[:d, _OFFF[k]] = f[k]
    return wb, wf


def _pack_sel(f):
    """16 selector variants for 4-tile-packed L1.  Variant v (tiles at xg rows
    4v..4v+3): [64, 128] with W1 in row 4v+j, cols 32j:32j+32."""
    sel = np.zeros((64, 16 * 128), NPBF)
    for v in range(16):
        for j in range(4):
            sel[4 * v + j, v * 128 + 32 * j: v * 128 + 32 * j + 32] = \
                f["W1"][0].astype(NPBF)
    return sel


# ------------------------------------------------------------- device build --
def _build(G):
    NT = G // 512            # 512-node tiles per group
    NXG = -(-NT // 64)       # 64-row x tiles per group
    NSLOT = 8 * NT           # slot columns per group (<= 512)
    NQ = -(-NSLOT // 128)    # stage-2 quarters
    assert G % 2048 == 0 and NSLOT <= 512

    nc = bacc.Bacc(None, target_bir_lowering=False)
    xs_d = nc.declare_dram_parameter("xs", [GROUPS, NXG, 64, 512], BF16,
                                     isOutput=False)
    sp_d = nc.declare_dram_parameter("sp", [GROUPS, NT // 4, 128, 32], BF16,
                                     isOutput=False)
    oh2_d = nc.declare_dram_parameter("oh2", [GROUPS, NQ, 128, BINS], BF16,
                                      isOutput=False)
    inv_d = nc.declare_dram_parameter("invbc", [128, GROUPS * BINS], F32,
                                      isOutput=False)
    wb_d = nc.declare_dram_parameter("wbf", [128, _CWB], BF16, isOutput=False)
    wf_d = nc.declare_dram_parameter("wfp", [128, _CWF], F32, isOutput=False)
    sel_d = nc.declare_dram_parameter("selc", [64, 16 * 128], BF16,
                                      isOutput=False)
    tab_d = nc.declare_dram_parameter("tab", [NROWS, 128], BF16,
                                      isOutput=False)
    idx_d = nc.declare_dram_parameter("idxg", [128, GROUPS, MAXGP, 64],
                                      I16, isOutput=False)
    out_d = nc.declare_dram_parameter("out", [NCLS, GROUPS * BINS], F32,
                                      isOutput=True)

    with ExitStack() as ctx:
        tc = ctx.enter_context(tile.TileContext(nc))
        cpool = ctx.enter_context(tc.tile_pool(name="const", bufs=1))
        gpool = ctx.enter_context(tc.tile_pool(name="gacc", bufs=1))
        xpool = ctx.enter_context(tc.tile_pool(name="xg", bufs=2))
        zpool = ctx.enter_context(tc.tile_pool(name="z", bufs=4))
        spool = ctx.enter_context(tc.tile_pool(name="small", bufs=8))
        psP = ctx.enter_context(tc.tile_pool(name="psP", bufs=4, space="PSUM"))
        psB = ctx.enter_context(tc.tile_pool(name="psB", bufs=3, space="PSUM"))
        psS = ctx.enter_context(tc.tile_pool(name="psS", bufs=1, space="PSUM"))

        nc.gpsimd.load_library(library_config.mlp)
        gbpool = ctx.enter_context(tc.tile_pool(name="gb", bufs=5))
        idxsb = cpool.tile([128, GROUPS, MAXGP, 64], I16)
        nc.sync.dma_start(idxsb[:], idx_d[:])
        wbsb = cpool.tile([128, _CWB], BF16)
        nc.sync.dma_start(wbsb[:], wb_d[:])
        wfsb = cpool.tile([128, _CWF], F32)
        nc.sync.dma_start(wfsb[:], wf_d[:])
        invsb = cpool.tile([128, GROUPS * BINS], F32)
        nc.sync.dma_start(invsb[:], inv_d[:])
        selsb = cpool.tile([64, 16 * 128], BF16)
        nc.sync.dma_start(selsb[:], sel_d[:])
        oh2sb = cpool.tile([128, GROUPS, NQ, BINS], BF16)
        for g in range(GROUPS):
            for q in range(NQ):
                nc.sync.dma_start(oh2sb[:, g, q, :], oh2_d[g, q])

        def WB(name, k, m):
            o = _OFFB[name]
            return wbsb[0:k, o:o + m]

        def WF(name, k, m):
            o = _OFFF[name]
            return wfsb[0:k, o:o + m]

        w2d = WB("W2D", 128, 128)
        w3d, w4d = WB("W3D", 128, 128), WB("W4D", 128, 128)
        w5r, w6 = WB("W5R", 128, 128), WB("W6", 128, 128)
        ones = WB("ONES", 128, 128)
        b6r4 = WB("B6R4", 128, 512)
        b6bc = WB("B6BC", 128, 512)
        f1, f2, f3 = WF("F1", 128, 64), WF("F2", 64, 32), WF("F3", 32, NCLS)
        idn = WF("IDN", 128, 128)
        b6p = WF("B6P", 128, 512)
        b1q, b2s = WF("B1Q", 128, 1), WF("B2S", 128, 1)
        b3s, b4s, b5 = WF("B3S", 128, 1), WF("B4S", 128, 1), WF("B5", 128, 1)
        f1b, f2b, f3b = WF("F1B", 64, 1), WF("F2B", 32, 1), WF("F3B", NCLS, 1)

        gsb = gpool.tile([128, GROUPS * BINS], F32)

        def z6phase(g, mi, z5c, spt, pslot, half):
            """L6 (ones prefill + 4 accumulating blocks) + ReLU evac; slot
            matmuls are deferred (returned as a pending item)."""
            p6 = psB.tile([128, 512], F32, tag="bg", name=f"p6_{g}_{mi}_{half}")
            nc.tensor.matmul(p6[:], ones, b6r4, start=True, stop=False,
                             skip_group_check=True)
            for s in range(4):
                nc.tensor.matmul(p6[:, s * 128:(s + 1) * 128],
                                 z5c[:, s * 128:(s + 1) * 128], w6,
                                 start=False, stop=(s == 3),
                                 skip_group_check=True)
            z6q = spool.tile([128, 512], BF16, tag="z6q",
                             name=f"z6q_{g}_{mi}_{half}")
            if (half + mi) % 2 == 0:
                nc.scalar.activation(z6q[:], p6[:], RELU)
            else:
                nc.vector.tensor_scalar(z6q[:], p6[:], 0.0, None, ALU.max)
            return (2 * mi + half, z6q, spt, pslot)

        def slotphase(item):
            t, z6q, spt, pslot = item
            u = t % 4                      # tile index within super-macro
            for s in range(4):
                sc = 8 * t + 2 * s
                spcol = 8 * u + 2 * s
                nc.tensor.matmul(pslot[:, sc:sc + 2],
                                 z6q[:, s * 128:(s + 1) * 128],
                                 spt[:, spcol:spcol + 2],
                                 start=True, stop=True,
                                 skip_group_check=True)

        def stage2(g, pslot):
            slots = spool.tile([128, 512], F32, tag="slots", name=f"slots{g}")
            nc.scalar.activation(slots[:, 0:NSLOT], pslot[:, 0:NSLOT],
                                 mybir.ActivationFunctionType.Copy)
            pT = psP.tile([128, 512], F32, tag="pk", name=f"pT{g}")
            for q in range(NQ):
                nc.tensor.transpose(pT[:, 128 * q:128 * q + 128],
                                    slots[:, 128 * q:128 * q + 128], idn)
            slotsT = spool.tile([128, 512], BF16, tag="slotsT",
                                name=f"slotsT{g}")
            nc.vector.tensor_scalar(slotsT[:, 0:128 * NQ], pT[:, 0:128 * NQ],
                                    0.0, None, ALU.add)
            pg = psP.tile([128, BINS], F32, tag="pk", name=f"pg{g}")
            for q in range(NQ):
                nc.tensor.matmul(pg[:], slotsT[:, 128 * q:128 * q + 128],
                                 oh2sb[:, g, q, :],
                                 start=(q == 0), stop=(q == NQ - 1),
                                 skip_group_check=True)
            nc.vector.tensor_tensor(gsb[:, g * BINS:(g + 1) * BINS], pg[:],
                                    invsb[:, g * BINS:(g + 1) * BINS],
                                    ALU.mult)

        def gatherphase(g, k, idxsb, pslot, spref):
            """Issue gather for tile pair (NTP+2k, NTP+2k+1) of group g."""
            gb = gbpool.tile([128, 8, 128], BF16, tag=f"gb{k % 5}",
                             name=f"gb{g}_{k}")
            nc.gpsimd.dma_gather(gb[:], tab_d[:], idxsb[:, g, k, :],
                                 1024, 1024, 128)
            return gb

        def gslotphase(g, k, gb, pslot, spg, NTP):
            for j in range(2):
                t = NTP + 2 * k + j
                u = t % 4
                for s in range(4):
                    sc = 8 * t + 2 * s
                    spcol = 8 * u + 2 * s
                    nc.tensor.matmul(pslot[:, sc:sc + 2],
                                     gb[:, 4 * j + s, :],
                                     spg[:, spcol:spcol + 2],
                                     start=True, stop=True,
                                     skip_group_check=True)

        prev = None
        pending = []
        pend_stage2 = None
        for g in range(GROUPS):
            NTG = NTGS[g]
            NTP = NT - NTG       # PE-pipeline tiles in this group
            assert NTP % 2 == 0 and NTG % 2 == 0
            xgs = {}
            for i in range(NXG):
                xg = xpool.tile([64, 512], BF16, tag=f"xg{i}",
                                name=f"xg{g}_{i}")
                nc.sync.dma_start(xg[:], xs_d[g, i])
                xgs[i] = xg

            pslot = psS.tile([128, 512], F32, tag="pslot", name=f"pslot{g}")
            z1cur = None

            # splitoh tiles covering the gather range
            spgs = {}
            for sm in range(NTP // 4, NT // 4):
                spg = spool.tile([128, 32], BF16, tag="spg",
                                 name=f"spg{g}_{sm}")
                nc.sync.dma_start(spg[:], sp_d[g, sm])
                spgs[sm] = spg
            gpend = []

            for mi in range(NTP // 2):
                # splitoh for the super-macro, loaded on its first macro
                if mi % 2 == 0:
                    spt = spool.tile([128, 32], BF16, tag="sp",
                                     name=f"sp{g}_{mi // 2}")
                    nc.sync.dma_start(spt[:], sp_d[g, mi // 2])

                    # ---- L1: one selector matmul -> 4 tiles of z1 ----
                    sm = mi // 2
                    v = sm % 16
                    p1 = psP.tile([128, 512], F32, tag="pk")
                    nc.tensor.matmul(p1[:], selsb[:, v * 128:(v + 1) * 128],
                                     xgs[(4 * sm) // 64][:],
                                     start=True, stop=True)
                    z1cur = zpool.tile([128, 512], BF16, tag="z1")
                    nc.scalar.activation(z1cur[:], p1[:], RELU, bias=b1q)
                z1h = z1cur[0:64, :] if mi % 2 == 0 else z1cur[64:128, :]

                # ---- L2: blockdiag K=64 (2 tiles) ----
                p2 = psP.tile([128, 512], F32, tag="pk")
                h = 0 if mi % 2 == 0 else 64
                nc.tensor.matmul(p2[:], w2d[h:h + 64, :], z1h,
                                 start=True, stop=True,
                                 tile_position=(h, 0))
                z2 = zpool.tile([128, 512], BF16, tag="z2")
                if mi % 2 == 0:
                    nc.scalar.activation(z2[:], p2[:], RELU, bias=b2s)
                else:
                    nc.vector.tensor_scalar(z2[:], p2[:], b2s, 0.0, ALU.add,
                                            ALU.max)

                # ---- L3: blockdiag K=128 ----
                p3 = psP.tile([128, 512], F32, tag="pk")
                nc.tensor.matmul(p3[:], w3d, z2[:], start=True, stop=True)
                z3 = zpool.tile([128, 512], BF16, tag="z3")
                nc.scalar.activation(z3[:], p3[:], RELU, bias=b3s)

                # ---- deferred L6 of the previous macro (half 0) ----
                if prev is not None:
                    g_, mi_, z5u_, z5v_, spt_, pslot_ = prev
                    pending.append(z6phase(g_, mi_, z5u_, spt_, pslot_, 0))
                    if pend_stage2 is not None:
                        stage2(*pend_stage2)
                        pend_stage2 = None

                # ---- L4 ----
                p4 = psP.tile([128, 512], F32, tag="pk")
                nc.tensor.matmul(p4[:], w4d, z3[:], start=True, stop=True)
                z4 = zpool.tile([128, 512], BF16, tag="z4")
                nc.vector.tensor_scalar(z4[:], p4[:], b4s, 0.0, ALU.add,
                                        ALU.max)

                # ---- deferred L6 of the previous macro (half 1) ----
                if prev is not None:
                    g_, mi_, z5u_, z5v_, spt_, pslot_ = prev
                    pending.append(z6phase(g_, mi_, z5v_, spt_, pslot_, 1))
                    prev = None

                # ---- drain slot matmuls two macros back ----
                while len(pending) > 2:
                    slotphase(pending.pop(0))

                # ---- weave gather issue / gather slot matmuls ----
                step = max(1, (NTP // 2) // max(1, NTG // 2 + 1))
                if mi % step == 0:
                    k = mi // step
                    if k < NTG // 2:
                        gb = gatherphase(g, k, idxsb, pslot, None)
                        gpend.append((k, gb))
                    if len(gpend) > 4:
                        k_, gb_ = gpend.pop(0)
                        gslotphase(g, k_, gb_,
                                   pslot, spgs[(NTP + 2 * k_) // 4], NTP)

                # ---- L5: row-split pair ----
                p5u = psB.tile([128, 512], F32, tag="bg")
                nc.tensor.matmul(p5u[:], w5r[0:64, :], z4[0:64, :],
                                 start=True, stop=True, tile_position=(0, 0))
                p5v = psB.tile([128, 512], F32, tag="bg")
                nc.tensor.matmul(p5v[:], w5r[64:128, :], z4[64:128, :],
                                 start=True, stop=True, tile_position=(64, 0))
                z5u = zpool.tile([128, 512], BF16, tag="z5u")
                nc.scalar.activation(z5u[:], p5u[:], RELU, bias=b5)
                z5v = zpool.tile([128, 512], BF16, tag="z5v")
                nc.vector.tensor_scalar(z5v[:], p5v[:], b5, 0.0, ALU.add,
                                        ALU.max)

                prev = (g, mi, z5u, z5v, spt, pslot)

            for k_, gb_ in gpend:
                gslotphase(g, k_, gb_, pslot, spgs[(NTP + 2 * k_) // 4], NTP)
            gpend = []
            if prev is not None:
                g_, mi_, z5u_, z5v_, spt_, pslot_ = prev
                pending.append(z6phase(g_, mi_, z5u_, spt_, pslot_, 0))
                pending.append(z6phase(g_, mi_, z5v_, spt_, pslot_, 1))
                prev = None
            for item in pending:
                slotphase(item)
            pending = []

            pend_stage2_next = (g, pslot)
            if g == GROUPS - 1:
                # flush: last macro + stage2 of the last two groups
                if prev is not None:
                    g_, mi_, z5u_, z5v_, spt_, pslot_ = prev
                    pending.append(z6phase(g_, mi_, z5u_, spt_, pslot_, 0))
                    pending.append(z6phase(g_, mi_, z5v_, spt_, pslot_, 1))
                    prev = None
                for item in pending:
                    slotphase(item)
                pending = []
                if pend_stage2 is not None:
                    stage2(*pend_stage2)
                stage2(*pend_stage2_next)
            else:
                pend_stage2 = pend_stage2_next

        # ---- final graph MLP ----
        pf1 = psP.tile([64, 512], F32, tag="pk")
        nc.tensor.matmul(pf1[:], f1, gsb[:], start=True, stop=True)
        a1 = zpool.tile([64, 512], F32, tag="a1")
        nc.scalar.activation(a1[:], pf1[:], RELU, bias=f1b)
        pf2 = psP.tile([32, 512], F32, tag="pk")
        nc.tensor.matmul(pf2[:], f2, a1[:], start=True, stop=True)
        a2 = zpool.tile([32, 512], F32, tag="a2")
        nc.scalar.activation(a2[:], pf2[:], RELU, bias=f2b)
        pf3 = psP.tile([NCLS, 512], F32, tag="pk")
        nc.tensor.matmul(pf3[:], f3, a2[:], start=True, stop=True)
        osb = zpool.tile([NCLS, 512], F32, tag="osb")
        nc.vector.tensor_scalar(osb[:], pf3[:], f3b, None, ALU.add)
        nc.sync.dma_start(out_d[:], osb[:])

    nc.compile()
    return nc


# -------------------------------------------------------------------- entry --
def kernel(**inputs):
    global LAST_RESULT
    x = np.asarray(inputs["x"], np.float32)
    batch = np.asarray(inputs["batch"], np.int32)
    B = int(np.asarray(inputs["num_graphs"]))
    assert B == NCORES * GROUPS * BINS, f"unexpected num_graphs {B}"

    params = {k: np.asarray(v, np.float32) for k, v in inputs.items()
              if k not in ("x", "batch", "num_graphs")}
    f = _fold(params)

    bounds = np.searchsorted(batch, np.arange(0, B + 1, BINS))
    seg = bounds[1:] - bounds[:-1]
    counts = np.bincount(batch, minlength=B)
    inv = (1.0 / np.maximum(counts, 1)).astype(np.float32)
    assert counts.min() >= 128, "block-span-2 assumption violated"

    G = max(2048, int(-(-int(seg.max()) // 2048) * 2048))
    NT = G // 512
    SM = NT // 4
    NXG = -(-NT // 64)
    NSLOT = 8 * NT
    NQ = -(-NSLOT // 128)
    assert NSLOT <= 512

    xs = np.zeros((NCORES, GROUPS, G), np.float32)
    bi = np.full((NCORES, GROUPS, G), -1, np.int64)
    for c in range(NCORES):
        for g in range(GROUPS):
            k = c * GROUPS + g
            s, e = int(bounds[k]), int(bounds[k + 1])
            n = e - s
            xs[c, g, :n] = x[s:e]
            bi[c, g, :n] = (batch[s:e] - k * BINS).astype(np.int64)
    xsp = np.zeros((NCORES, GROUPS, NXG * 64, 512), np.float32)
    xsp[:, :, :NT] = xs.reshape(NCORES, GROUPS, NT, 512)
    xsp = xsp.reshape(NCORES, GROUPS, NXG, 64, 512).astype(NPBF)

    # split one-hot: block (t, s) of 128 nodes spans <= 2 graphs.
    # run 0 = first graph of the block, run 1 = second (if present).
    biB = bi.reshape(NCORES, GROUPS, NT, 4, 128)  # [c, g, t, s, node]
    first = biB[..., 0]                            # bin of node 0 (or -1)
    # a padded block ([-1...]) contributes nothing
    firstv = np.where(first < 0, 0, first)
    isfirst = (biB == firstv[..., None])
    valid = biB >= 0
    run0 = (isfirst & valid).astype(NPBF)          # [c,g,t,s,128]
    run1 = ((~isfirst) & valid).astype(NPBF)
    # sp layout: [c, g, sm, node(128), 32] with col 8*u + 2*s + r for
    # tile-in-sm u, block s, run r
    sp = np.zeros((NCORES, GROUPS, SM, 128, 32), NPBF)
    r0 = run0.transpose(0, 1, 2, 4, 3)             # [c,g,t,node,s]
    r1 = run1.transpose(0, 1, 2, 4, 3)
    for u in range(4):
        tsel = np.arange(SM) * 4 + u
        sp[:, :, :, :, 8 * u + 0:8 * u + 8:2] = r0[:, :, tsel]
        sp[:, :, :, :, 8 * u + 1:8 * u + 8:2] = r1[:, :, tsel]

    # slot -> bin map: slot 8t+2s+r of group g -> bin value
    second = np.where(valid & ~isfirst, biB, -1).max(axis=-1)  # [c,g,t,s]
    firstbin = np.where(valid.any(axis=-1), firstv, -1)
    slotbin = np.stack([firstbin, second], axis=-1)  # [c,g,t,s,2]
    slotbin = slotbin.reshape(NCORES, GROUPS, NSLOT)
    oh2 = np.zeros((NCORES, GROUPS, NQ * 128, BINS), NPBF)
    cc, gg, ss = np.nonzero(slotbin >= 0)
    oh2[cc, gg, ss, slotbin[cc, gg, ss]] = NPBF(1.0)
    oh2 = oh2.reshape(NCORES, GROUPS, NQ, 128, BINS)

    invbc = np.ascontiguousarray(
        np.broadcast_to(inv.reshape(NCORES, GROUPS * BINS)[:, None, :],
                        (NCORES, 128, GROUPS * BINS)))

    wb, wf = _pack_consts(f)
    sel = _pack_sel(f)

    # ---- z6 lookup table over the x quantization grid ----
    grid = (XLO + XSTEP * np.arange(NROWS)).astype(np.float32)
    relu = lambda a: np.maximum(a, 0.0, out=a)
    tz = relu(grid[:, None] * f["W1"][0][None, :] + f["B1"])
    tz = relu(tz @ f["W2"] + f["B2"])
    tz = relu(tz @ f["W3"] + f["B3"])
    tz = relu(tz @ f["W4"] + f["B4"])
    tz = relu(tz @ f["W5"] + f["B5"])
    tz = relu(tz @ f["W6"] + f["B6"])
    tab = tz.astype(NPBF)                       # [NROWS, 128]

    # gather indices for the tail NTG tiles of each group
    q = np.clip(np.rint((xs - XLO) / XSTEP), 0, NROWS - 1).astype(np.int64)
    # device layout: [128, GROUPS, MAXGP, 64]; idx i -> [16c + i%16, i//16]
    idxg = np.zeros((NCORES, 128, GROUPS, MAXGP, 64), np.int16)
    for g in range(GROUPS):
        NTGg = NTGS[g]
        NTPg = NT - NTGg
        qg = q[:, g, NTPg * 512:].reshape(NCORES, NTGg // 2, 1024)
        wrap = qg.astype(np.int16).reshape(NCORES, NTGg // 2, 64, 16)
        wrap = wrap.transpose(0, 3, 1, 2)       # [c, 16, K, 64]
        idxg[:, :, g, :NTGg // 2] = np.tile(wrap, (1, 8, 1, 1))

    if G not in _NC_CACHE:
        _NC_CACHE[G] = _build(G)
    nc = _NC_CACHE[G]

    in_maps = [{"xs": xsp[c], "sp": sp[c], "oh2": oh2[c], "invbc": invbc[c],
                "wbf": wb, "wfp": wf, "selc": sel, "tab": tab,
                "idxg": idxg[c]} for c in range(NCORES)]
    res = run_bass_kernel_spmd(nc, in_maps, core_ids=list(range(NCORES)))
    LAST_RESULT = res
    outs = np.stack([res.results[i]["out"] for i in range(NCORES)])
    return np.ascontiguousarray(
        outs.transpose(0, 2, 1).reshape(B, NCLS)).astype(np.float32)


# revision 40
# speedup vs baseline: 1.1570x; 1.1570x over previous
"""Trainium2 Bass kernel for AdaptedEnzymeModel (per-node MLP -> segment mean
pool -> graph MLP), SPMD over 8 NeuronCores.

Strategy (hybrid PE pipeline + table gather)
--------------------------------------------
* BN affines folded into adjacent Linears on host; device runs Linear+ReLU
  chains in bf16 (fp32 PSUM).
* Node dim sharded at graph boundaries: core c owns graphs [512c, 512c+512),
  4 groups of 128 graphs, each padded to G nodes (multiple of 2048).
* Because the node MLP has input dimension 1, z6 = MLP(x) is a function of a
  single scalar.  The host tabulates it on a 32768-level grid over x
  (step 1/2048, quantization error ~2e-4) as a [32768, 128] bf16 table.
  The last NTG tiles of each group are produced by SWDGE dma_gather
  (256B rows keyed by the quantized x) on the otherwise-idle GpSimd engine,
  bypassing the PE and both evacuation engines entirely for ~25% of nodes.
* The remaining tiles run the PE pipeline, 2-tile macros with the proven
  deferred-phase overlap:
  - L1 is an outer product: one K=64 selector matmul computes four tiles at
    once into one [128 = 4x32 feats, 512] bank (amortized over 2 macros).
  - L2: block-diagonal K=64 weights at row position 0/64 per macro parity.
  - L3/L4: 2-tile block-diagonal K=128 matmuls (one per macro).
  - L5 (K=64/tile, M=128): row-split pairs at positions (0,0)/(64,0), which
    stream concurrently on disjoint PE row bands.
  - L6 in [node, feat] layout: ones-row bias prefill + 4 K=128 block
    matmuls per tile, deferred one macro so its PE work hides evacuations.
* Segment mean-pool without one-hot streams: every 128-node block spans at
  most 2 graphs (min graph size 192), so a per-tile [128, 8] "split one-hot"
  (2 run columns per block) turns the pooling into N=2 slot matmuls that
  accumulate into a per-group [128, 8*NT] slot PSUM bank; gathered tiles
  feed the same slot matmuls.  Stage 2 (per group): evacuate slots,
  PE-transpose via identity, multiply by a [slots, bins] one-hot to get
  per-graph sums, scale by exact fp32 1/count.
* Evacuations are balanced across Scalar and Vector by phase/parity.
* Final graph MLP (128->64->32->7) on-device.
"""

import numpy as np
import ml_dtypes
from contextlib import ExitStack

import concourse.bass as bass
import concourse.tile as tile
from concourse import bacc, mybir, library_config
from concourse.bass_utils import run_bass_kernel_spmd

NCORES = 8
GROUPS = 4          # bin-groups per core
BINS = 128          # graphs per group
NCLS = 7
EPS = 1e-5
F32 = mybir.dt.float32
BF16 = mybir.dt.bfloat16
NPBF = ml_dtypes.bfloat16
RELU = mybir.ActivationFunctionType.Relu
ALU = mybir.AluOpType

LAST_RESULT = None
_NC_CACHE = {}
I16 = mybir.dt.int16
NTGS = (14, 16, 20, 22)   # gather-sourced tiles per group (later groups
                          # lean on the gather queue that outlives their PE)
MAXGP = max(NTGS) // 2
XLO, XSTEP = -8.0, 1.0 / 2048.0   # x quantization grid for the z6 table
NROWS = 32768


def _ensure_ntff_hook():
    """bass_utils' trace path needs antenv.axon_hooks, which this image's
    antenv package lacks.  Register a shim backed by trn_agent_boot's ctypes
    NTFF driver so BASS_TRACE=1 yields exec_time_ns.  Degrades silently."""
    import sys
    import types
    try:
        import antenv
        if "antenv.axon_hooks" in sys.modules:
            return
        mod = types.ModuleType("antenv.axon_hooks")
        mod._hook = None
        mod.set_axon_ntff_profile_hook = lambda h: setattr(mod, "_hook", h)
        mod.get_axon_ntff_profile_hook = lambda: mod._hook
        sys.modules["antenv.axon_hooks"] = mod
        antenv.axon_hooks = mod
        from trn_agent_boot.trn_boot import _ntff_profile_via_ctypes
        mod._hook = _ntff_profile_via_ctypes("/opt/axon/libaxon_pjrt.so")
    except Exception:
        pass


_ensure_ntff_hook()


# ---------------------------------------------------------------- host math --
def _fold(p):
    """Fold eval-mode BN affines into adjacent linears. Returns dict of f32."""
    def aff(bn):
        g, b, m, v = bn[0], bn[1], bn[2], bn[3]
        s = g / np.sqrt(v + EPS)
        return s.astype(np.float32), (b - m * s).astype(np.float32)

    s1, t1 = aff(p["ne_bn1"]); s2, t2 = aff(p["ne_bn2"])
    sc1, tc1 = aff(p["cbn1"]); sc2, tc2 = aff(p["cbn2"])
    sf1, tf1 = aff(p["fbn1"]); sf2, tf2 = aff(p["fbn2"])
    f = {}
    f["W1"] = p["ne_w1"]; f["B1"] = p["ne_b1"]
    f["W2"] = s1[:, None] * p["ne_w2"]; f["B2"] = t1 @ p["ne_w2"] + p["ne_b2"]
    f["W3"] = s2[:, None] * p["c1a_w"]; f["B3"] = t2 @ p["c1a_w"] + p["c1a_b"]
    f["W4"] = p["c1b_w"];               f["B4"] = p["c1b_b"]
    f["W5"] = sc1[:, None] * p["c2a_w"]; f["B5"] = tc1 @ p["c2a_w"] + p["c2a_b"]
    f["W6"] = p["c2b_w"];               f["B6"] = p["c2b_b"]
    f["F1"] = sc2[:, None] * p["f1_w"]; f["F1B"] = tc2 @ p["f1_w"] + p["f1_b"]
    f["F2"] = sf1[:, None] * p["f2_w"]; f["F2B"] = tf1 @ p["f2_w"] + p["f2_b"]
    f["F3"] = sf2[:, None] * p["f3_w"]; f["F3B"] = tf2 @ p["f3_w"] + p["f3_b"]
    return {k: np.asarray(v, np.float32) for k, v in f.items()}


# bf16 const block layout
def _layout_bf():
    off, c = {}, 0
    for name, ncols in [("W2D", 128),    # blockdiag(W2pad, W2pad) K=64, M=128
                        ("W3D", 128),    # blockdiag(W3, W3) K=128, M=128
                        ("W4D", 128),
                        ("W5R", 128),    # W5 replicated on rows 0:64 / 64:128
                        ("W6", 128),
                        ("ONES", 128),
                        ("B6R4", 512), ("B6BC", 512)]:
        off[name] = c
        c += ncols
    return off, c


# f32 const block: biases + final mlp + identity (for PE transpose)
def _layout_fp():
    off, c = {}, 0
    for name, ncols in [("F1", 64), ("F2", 32), ("F3", NCLS), ("IDN", 128),
                        ("B6P", 512),
                        ("B1Q", 1),   # B1 tiled x4 (4-tile packed z1)
                        ("B2S", 1), ("B3S", 1), ("B4S", 1),
                        ("B5", 1), ("F1B", 1), ("F2B", 1), ("F3B", 1)]:
        off[name] = c
        c += ncols
    return off, c


_OFFB, _CWB = _layout_bf()
_OFFF, _CWF = _layout_fp()


def _pack_consts(f):
    wb = np.zeros((128, _CWB), NPBF)

    def putb(name, arr):
        wb[:arr.shape[0], _OFFB[name]:_OFFB[name] + arr.shape[1]] = \
            arr.astype(NPBF)

    # W2D stacked twice so both (0,0) and (64,0) row positions can read it
    w2d = np.zeros((128, 128), np.float32)
    for h in (0, 64):
        w2d[h + 0:h + 32, 0:64] = f["W2"]
        w2d[h + 32:h + 64, 64:128] = f["W2"]
    putb("W2D", w2d)
    w3d = np.zeros((128, 128), np.float32)
    w3d[0:64, 0:64] = f["W3"]
    w3d[64:128, 64:128] = f["W3"]
    putb("W3D", w3d)
    w4d = np.zeros((128, 128), np.float32)
    w4d[0:64, 0:64] = f["W4"]
    w4d[64:128, 64:128] = f["W4"]
    putb("W4D", w4d)
    putb("W5R", np.tile(f["W5"], (2, 1)))
    putb("W6", f["W6"])
    wb[0, _OFFB["ONES"]:_OFFB["ONES"] + 128] = NPBF(1.0)
    wb[0, _OFFB["B6R4"]:_OFFB["B6R4"] + 512] = np.tile(f["B6"].astype(NPBF), 4)
    wb[:, _OFFB["B6BC"]:_OFFB["B6BC"] + 512] = \
        np.tile(f["B6"], 4)[None, :].astype(NPBF)

    wf = np.zeros((128, _CWF), np.float32)
    for k in ["F1", "F2", "F3"]:
        arr = f[k]
        wf[:arr.shape[0], _OFFF[k]:_OFFF[k] + arr.shape[1]] = arr
    wf[:, _OFFF["IDN"]:_OFFF["IDN"] + 128] = np.eye(128, dtype=np.float32)
    wf[:, _OFFF["B6P"]:_OFFF["B6P"] + 512] = np.tile(f["B6"], 4)[None, :]
    wf[:, _OFFF["B1Q"]] = np.tile(f["B1"], 4)
    wf[:, _OFFF["B2S"]] = np.tile(f["B2"], 2)
    wf[:, _OFFF["B3S"]] = np.tile(f["B3"], 2)
    wf[:, _OFFF["B4S"]] = np.tile(f["B4"], 2)
    wf[:128, _OFFF["B5"]] = f["B5"]
    for k, d in [("F1B", 64), ("F2B", 32), ("F3B", NCLS)]:
        wf[:d, _OFFF[k]] = f[k]
    return wb, wf


def _pack_sel(f):
    """16 selector variants for 4-tile-packed L1.  Variant v (tiles at xg rows
    4v..4v+3): [64, 128] with W1 in row 4v+j, cols 32j:32j+32."""
    sel = np.zeros((64, 16 * 128), NPBF)
    for v in range(16):
        for j in range(4):
            sel[4 * v + j, v * 128 + 32 * j: v * 128 + 32 * j + 32] = \
                f["W1"][0].astype(NPBF)
    return sel


# ------------------------------------------------------------- device build --
def _build(G):
    NT = G // 512            # 512-node tiles per group
    NXG = -(-NT // 64)       # 64-row x tiles per group
    NSLOT = 8 * NT           # slot columns per group (<= 512)
    NQ = -(-NSLOT // 128)    # stage-2 quarters
    assert G % 2048 == 0 and NSLOT <= 512

    nc = bacc.Bacc(None, target_bir_lowering=False)
    xs_d = nc.declare_dram_parameter("xs", [GROUPS, NXG, 64, 512], BF16,
                                     isOutput=False)
    sp_d = nc.declare_dram_parameter("sp", [GROUPS, NT // 4, 128, 32], BF16,
                                     isOutput=False)
    oh2_d = nc.declare_dram_parameter("oh2", [GROUPS, NQ, 128, BINS], BF16,
                                      isOutput=False)
    inv_d = nc.declare_dram_parameter("invbc", [128, GROUPS * BINS], F32,
                                      isOutput=False)
    wb_d = nc.declare_dram_parameter("wbf", [128, _CWB], BF16, isOutput=False)
    wf_d = nc.declare_dram_parameter("wfp", [128, _CWF], F32, isOutput=False)
    sel_d = nc.declare_dram_parameter("selc", [64, 16 * 128], BF16,
                                      isOutput=False)
    tab_d = nc.declare_dram_parameter("tab", [NROWS, 128], BF16,
                                      isOutput=False)
    idx_d = nc.declare_dram_parameter("idxg", [128, GROUPS, MAXGP, 64],
                                      I16, isOutput=False)
    out_d = nc.declare_dram_parameter("out", [NCLS, GROUPS * BINS], F32,
                                      isOutput=True)

    with ExitStack() as ctx:
        tc = ctx.enter_context(tile.TileContext(nc))
        cpool = ctx.enter_context(tc.tile_pool(name="const", bufs=1))
        gpool = ctx.enter_context(tc.tile_pool(name="gacc", bufs=1))
        xpool = ctx.enter_context(tc.tile_pool(name="xg", bufs=2))
        zpool = ctx.enter_context(tc.tile_pool(name="z", bufs=4))
        spool = ctx.enter_context(tc.tile_pool(name="small", bufs=8))
        psP = ctx.enter_context(tc.tile_pool(name="psP", bufs=4, space="PSUM"))
        psB = ctx.enter_context(tc.tile_pool(name="psB", bufs=3, space="PSUM"))
        psS = ctx.enter_context(tc.tile_pool(name="psS", bufs=1, space="PSUM"))

        nc.gpsimd.load_library(library_config.mlp)
        gbpool = ctx.enter_context(tc.tile_pool(name="gb", bufs=5))
        idxsb = cpool.tile([128, GROUPS, MAXGP, 64], I16)
        nc.sync.dma_start(idxsb[:], idx_d[:])
        wbsb = cpool.tile([128, _CWB], BF16)
        nc.sync.dma_start(wbsb[:], wb_d[:])
        wfsb = cpool.tile([128, _CWF], F32)
        nc.sync.dma_start(wfsb[:], wf_d[:])
        invsb = cpool.tile([128, GROUPS * BINS], F32)
        nc.sync.dma_start(invsb[:], inv_d[:])
        selsb = cpool.tile([64, 16 * 128], BF16)
        nc.sync.dma_start(selsb[:], sel_d[:])
        oh2sb = cpool.tile([128, GROUPS, NQ, BINS], BF16)
        for g in range(GROUPS):
            for q in range(NQ):
                nc.sync.dma_start(oh2sb[:, g, q, :], oh2_d[g, q])

        def WB(name, k, m):
            o = _OFFB[name]
            return wbsb[0:k, o:o + m]

        def WF(name, k, m):
            o = _OFFF[name]
            return wfsb[0:k, o:o + m]

        w2d = WB("W2D", 128, 128)
        w3d, w4d = WB("W3D", 128, 128), WB("W4D", 128, 128)
        w5r, w6 = WB("W5R", 128, 128), WB("W6", 128, 128)
        ones = WB("ONES", 128, 128)
        b6r4 = WB("B6R4", 128, 512)
        b6bc = WB("B6BC", 128, 512)
        f1, f2, f3 = WF("F1", 128, 64), WF("F2", 64, 32), WF("F3", 32, NCLS)
        idn = WF("IDN", 128, 128)
        b6p = WF("B6P", 128, 512)
        b1q, b2s = WF("B1Q", 128, 1), WF("B2S", 128, 1)
        b3s, b4s, b5 = WF("B3S", 128, 1), WF("B4S", 128, 1), WF("B5", 128, 1)
        f1b, f2b, f3b = WF("F1B", 64, 1), WF("F2B", 32, 1), WF("F3B", NCLS, 1)

        gsb = gpool.tile([128, GROUPS * BINS], F32)

        def z6phase(g, mi, z5c, spt, pslot, half):
            """L6 (ones prefill + 4 accumulating blocks) + ReLU evac; slot
            matmuls are deferred (returned as a pending item)."""
            p6 = psB.tile([128, 512], F32, tag="bg", name=f"p6_{g}_{mi}_{half}")
            nc.tensor.matmul(p6[:], ones, b6r4, start=True, stop=False,
                             skip_group_check=True)
            for s in range(4):
                nc.tensor.matmul(p6[:, s * 128:(s + 1) * 128],
                                 z5c[:, s * 128:(s + 1) * 128], w6,
                                 start=False, stop=(s == 3),
                                 skip_group_check=True)
            z6q = spool.tile([128, 512], BF16, tag="z6q",
                             name=f"z6q_{g}_{mi}_{half}")
            if (half + mi) % 2 == 0:
                nc.scalar.activation(z6q[:], p6[:], RELU)
            else:
                nc.vector.tensor_scalar(z6q[:], p6[:], 0.0, None, ALU.max)
            return (2 * mi + half, z6q, spt, pslot)

        def slotphase(item):
            t, z6q, spt, pslot = item
            u = t % 4                      # tile index within super-macro
            for s in range(4):
                sc = 8 * t + 2 * s
                spcol = 8 * u + 2 * s
                nc.tensor.matmul(pslot[:, sc:sc + 2],
                                 z6q[:, s * 128:(s + 1) * 128],
                                 spt[:, spcol:spcol + 2],
                                 start=True, stop=True,
                                 skip_group_check=True)

        def stage2(g, pslot):
            slots = spool.tile([128, 512], F32, tag="slots", name=f"slots{g}")
            nc.scalar.activation(slots[:, 0:NSLOT], pslot[:, 0:NSLOT],
                                 mybir.ActivationFunctionType.Copy)
            pT = psP.tile([128, 512], F32, tag="pk", name=f"pT{g}")
            for q in range(NQ):
                nc.tensor.transpose(pT[:, 128 * q:128 * q + 128],
                                    slots[:, 128 * q:128 * q + 128], idn)
            slotsT = spool.tile([128, 512], BF16, tag="slotsT",
                                name=f"slotsT{g}")
            nc.vector.tensor_scalar(slotsT[:, 0:128 * NQ], pT[:, 0:128 * NQ],
                                    0.0, None, ALU.add)
            pg = psP.tile([128, BINS], F32, tag="pk", name=f"pg{g}")
            for q in range(NQ):
                nc.tensor.matmul(pg[:], slotsT[:, 128 * q:128 * q + 128],
                                 oh2sb[:, g, q, :],
                                 start=(q == 0), stop=(q == NQ - 1),
                                 skip_group_check=True)
            nc.vector.tensor_tensor(gsb[:, g * BINS:(g + 1) * BINS], pg[:],
                                    invsb[:, g * BINS:(g + 1) * BINS],
                                    ALU.mult)

        def gatherphase(g, k, idxsb, pslot, spref):
            """Issue gather for tile pair (NTP+2k, NTP+2k+1) of group g."""
            gb = gbpool.tile([128, 8, 128], BF16, tag=f"gb{k % 5}",
                             name=f"gb{g}_{k}")
            nc.gpsimd.dma_gather(gb[:], tab_d[:], idxsb[:, g, k, :],
                                 1024, 1024, 128)
            return gb

        def gslotphase(g, k, gb, pslot, spg, NTP):
            for j in range(2):
                t = NTP + 2 * k + j
                u = t % 4
                for s in range(4):
                    sc = 8 * t + 2 * s
                    spcol = 8 * u + 2 * s
                    nc.tensor.matmul(pslot[:, sc:sc + 2],
                                     gb[:, 4 * j + s, :],
                                     spg[:, spcol:spcol + 2],
                                     start=True, stop=True,
                                     skip_group_check=True)

        prev = None
        pending = []
        pend_stage2 = None
        for g in range(GROUPS):
            NTG = NTGS[g]
            NTP = NT - NTG       # PE-pipeline tiles in this group
            assert NTP % 2 == 0 and NTG % 2 == 0
            xgs = {}
            for i in range(NXG):
                xg = xpool.tile([64, 512], BF16, tag=f"xg{i}",
                                name=f"xg{g}_{i}")
                nc.sync.dma_start(xg[:], xs_d[g, i])
                xgs[i] = xg

            pslot = psS.tile([128, 512], F32, tag="pslot", name=f"pslot{g}")
            z1cur = None

            # splitoh tiles covering the gather range
            spgs = {}
            for sm in range(NTP // 4, NT // 4):
                spg = spool.tile([128, 32], BF16, tag="spg",
                                 name=f"spg{g}_{sm}")
                nc.sync.dma_start(spg[:], sp_d[g, sm])
                spgs[sm] = spg
            gpend = []

            for mi in range(NTP // 2):
                # splitoh for the super-macro, loaded on its first macro
                if mi % 2 == 0:
                    spt = spool.tile([128, 32], BF16, tag="sp",
                                     name=f"sp{g}_{mi // 2}")
                    nc.sync.dma_start(spt[:], sp_d[g, mi // 2])

                    # ---- L1: one selector matmul -> 4 tiles of z1 ----
                    sm = mi // 2
                    v = sm % 16
                    p1 = psP.tile([128, 512], F32, tag="pk")
                    nc.tensor.matmul(p1[:], selsb[:, v * 128:(v + 1) * 128],
                                     xgs[(4 * sm) // 64][:],
                                     start=True, stop=True)
                    z1cur = zpool.tile([128, 512], BF16, tag="z1")
                    nc.scalar.activation(z1cur[:], p1[:], RELU, bias=b1q)
                z1h = z1cur[0:64, :] if mi % 2 == 0 else z1cur[64:128, :]

                # ---- L2: blockdiag K=64 (2 tiles) ----
                p2 = psP.tile([128, 512], F32, tag="pk")
                h = 0 if mi % 2 == 0 else 64
                nc.tensor.matmul(p2[:], w2d[h:h + 64, :], z1h,
                                 start=True, stop=True,
                                 tile_position=(h, 0))
                z2 = zpool.tile([128, 512], BF16, tag="z2")
                if mi % 2 == 0:
                    nc.scalar.activation(z2[:], p2[:], RELU, bias=b2s)
                else:
                    nc.vector.tensor_scalar(z2[:], p2[:], b2s, 0.0, ALU.add,
                                            ALU.max)

                # ---- L3: blockdiag K=128 ----
                p3 = psP.tile([128, 512], F32, tag="pk")
                nc.tensor.matmul(p3[:], w3d, z2[:], start=True, stop=True)
                z3 = zpool.tile([128, 512], BF16, tag="z3")
                nc.scalar.activation(z3[:], p3[:], RELU, bias=b3s)

                # ---- deferred L6 of the previous macro (half 0) ----
                if prev is not None:
                    g_, mi_, z5u_, z5v_, spt_, pslot_ = prev
                    pending.append(z6phase(g_, mi_, z5u_, spt_, pslot_, 0))
                    if pend_stage2 is not None:
                        stage2(*pend_stage2)
                        pend_stage2 = None

                # ---- L4 ----
                p4 = psP.tile([128, 512], F32, tag="pk")
                nc.tensor.matmul(p4[:], w4d, z3[:], start=True, stop=True)
                z4 = zpool.tile([128, 512], BF16, tag="z4")
                nc.vector.tensor_scalar(z4[:], p4[:], b4s, 0.0, ALU.add,
                                        ALU.max)

                # ---- deferred L6 of the previous macro (half 1) ----
                if prev is not None:
                    g_, mi_, z5u_, z5v_, spt_, pslot_ = prev
                    pending.append(z6phase(g_, mi_, z5v_, spt_, pslot_, 1))
                    prev = None

                # ---- drain slot matmuls two macros back ----
                while len(pending) > 2:
                    slotphase(pending.pop(0))

                # ---- weave gather issue / gather slot matmuls ----
                step = max(1, (NTP // 2) // max(1, NTG // 2 + 1))
                if mi % step == 0:
                    k = mi // step
                    if k < NTG // 2:
                        gb = gatherphase(g, k, idxsb, pslot, None)
                        gpend.append((k, gb))
                    if len(gpend) > 3:
                        k_, gb_ = gpend.pop(0)
                        gslotphase(g, k_, gb_,
                                   pslot, spgs[(NTP + 2 * k_) // 4], NTP)

                # ---- L5: row-split pair ----
                p5u = psB.tile([128, 512], F32, tag="bg")
                nc.tensor.matmul(p5u[:], w5r[0:64, :], z4[0:64, :],
                                 start=True, stop=True, tile_position=(0, 0))
                p5v = psB.tile([128, 512], F32, tag="bg")
                nc.tensor.matmul(p5v[:], w5r[64:128, :], z4[64:128, :],
                                 start=True, stop=True, tile_position=(64, 0))
                z5u = zpool.tile([128, 512], BF16, tag="z5u")
                nc.scalar.activation(z5u[:], p5u[:], RELU, bias=b5)
                z5v = zpool.tile([128, 512], BF16, tag="z5v")
                nc.vector.tensor_scalar(z5v[:], p5v[:], b5, 0.0, ALU.add,
                                        ALU.max)

                prev = (g, mi, z5u, z5v, spt, pslot)

            for k_, gb_ in gpend:
                gslotphase(g, k_, gb_, pslot, spgs[(NTP + 2 * k_) // 4], NTP)
            gpend = []
            if prev is not None:
                g_, mi_, z5u_, z5v_, spt_, pslot_ = prev
                pending.append(z6phase(g_, mi_, z5u_, spt_, pslot_, 0))
                pending.append(z6phase(g_, mi_, z5v_, spt_, pslot_, 1))
                prev = None
            for item in pending:
                slotphase(item)
            pending = []

            pend_stage2_next = (g, pslot)
            if g == GROUPS - 1:
                # flush: last macro + stage2 of the last two groups
                if prev is not None:
                    g_, mi_, z5u_, z5v_, spt_, pslot_ = prev
                    pending.append(z6phase(g_, mi_, z5u_, spt_, pslot_, 0))
                    pending.append(z6phase(g_, mi_, z5v_, spt_, pslot_, 1))
                    prev = None
                for item in pending:
                    slotphase(item)
                pending = []
                if pend_stage2 is not None:
                    stage2(*pend_stage2)
                stage2(*pend_stage2_next)
            else:
                pend_stage2 = pend_stage2_next

        # ---- final graph MLP ----
        pf1 = psP.tile([64, 512], F32, tag="pk")
        nc.tensor.matmul(pf1[:], f1, gsb[:], start=True, stop=True)
        a1 = zpool.tile([64, 512], F32, tag="a1")
        nc.scalar.activation(a1[:], pf1[:], RELU, bias=f1b)
        pf2 = psP.tile([32, 512], F32, tag="pk")
        nc.tensor.matmul(pf2[:], f2, a1[:], start=True, stop=True)
        a2 = zpool.tile([32, 512], F32, tag="a2")
        nc.scalar.activation(a2[:], pf2[:], RELU, bias=f2b)
        pf3 = psP.tile([NCLS, 512], F32, tag="pk")
        nc.tensor.matmul(pf3[:], f3, a2[:], start=True, stop=True)
        osb = zpool.tile([NCLS, 512], F32, tag="osb")
        nc.vector.tensor_scalar(osb[:], pf3[:], f3b, None, ALU.add)
        nc.sync.dma_start(out_d[:], osb[:])

    nc.compile()
    return nc


# -------------------------------------------------------------------- entry --
def kernel(**inputs):
    global LAST_RESULT
    x = np.asarray(inputs["x"], np.float32)
    batch = np.asarray(inputs["batch"], np.int32)
    B = int(np.asarray(inputs["num_graphs"]))
    assert B == NCORES * GROUPS * BINS, f"unexpected num_graphs {B}"

    params = {k: np.asarray(v, np.float32) for k, v in inputs.items()
              if k not in ("x", "batch", "num_graphs")}
    f = _fold(params)

    bounds = np.searchsorted(batch, np.arange(0, B + 1, BINS))
    seg = bounds[1:] - bounds[:-1]
    counts = np.bincount(batch, minlength=B)
    inv = (1.0 / np.maximum(counts, 1)).astype(np.float32)
    assert counts.min() >= 128, "block-span-2 assumption violated"

    G = max(2048, int(-(-int(seg.max()) // 2048) * 2048))
    NT = G // 512
    SM = NT // 4
    NXG = -(-NT // 64)
    NSLOT = 8 * NT
    NQ = -(-NSLOT // 128)
    assert NSLOT <= 512

    xs = np.zeros((NCORES, GROUPS, G), np.float32)
    bi = np.full((NCORES, GROUPS, G), -1, np.int64)
    for c in range(NCORES):
        for g in range(GROUPS):
            k = c * GROUPS + g
            s, e = int(bounds[k]), int(bounds[k + 1])
            n = e - s
            xs[c, g, :n] = x[s:e]
            bi[c, g, :n] = (batch[s:e] - k * BINS).astype(np.int64)
    xsp = np.zeros((NCORES, GROUPS, NXG * 64, 512), np.float32)
    xsp[:, :, :NT] = xs.reshape(NCORES, GROUPS, NT, 512)
    xsp = xsp.reshape(NCORES, GROUPS, NXG, 64, 512).astype(NPBF)

    # split one-hot: block (t, s) of 128 nodes spans <= 2 graphs.
    # run 0 = first graph of the block, run 1 = second (if present).
    biB = bi.reshape(NCORES, GROUPS, NT, 4, 128)  # [c, g, t, s, node]
    first = biB[..., 0]                            # bin of node 0 (or -1)
    # a padded block ([-1...]) contributes nothing
    firstv = np.where(first < 0, 0, first)
    isfirst = (biB == firstv[..., None])
    valid = biB >= 0
    run0 = (isfirst & valid).astype(NPBF)          # [c,g,t,s,128]
    run1 = ((~isfirst) & valid).astype(NPBF)
    # sp layout: [c, g, sm, node(128), 32] with col 8*u + 2*s + r for
    # tile-in-sm u, block s, run r
    sp = np.zeros((NCORES, GROUPS, SM, 128, 32), NPBF)
    r0 = run0.transpose(0, 1, 2, 4, 3)             # [c,g,t,node,s]
    r1 = run1.transpose(0, 1, 2, 4, 3)
    for u in range(4):
        tsel = np.arange(SM) * 4 + u
        sp[:, :, :, :, 8 * u + 0:8 * u + 8:2] = r0[:, :, tsel]
        sp[:, :, :, :, 8 * u + 1:8 * u + 8:2] = r1[:, :, tsel]

    # slot -> bin map: slot 8t+2s+r of group g -> bin value
    second = np.where(valid & ~isfirst, biB, -1).max(axis=-1)  # [c,g,t,s]
    firstbin = np.where(valid.any(axis=-1), firstv, -1)
    slotbin = np.stack([firstbin, second], axis=-1)  # [c,g,t,s,2]
    slotbin = slotbin.reshape(NCORES, GROUPS, NSLOT)
    oh2 = np.zeros((NCORES, GROUPS, NQ * 128, BINS), NPBF)
    cc, gg, ss = np.nonzero(slotbin >= 0)
    oh2[cc, gg, ss, slotbin[cc, gg, ss]] = NPBF(1.0)
    oh2 = oh2.reshape(NCORES, GROUPS, NQ, 128, BINS)

    invbc = np.ascontiguousarray(
        np.broadcast_to(inv.reshape(NCORES, GROUPS * BINS)[:, None, :],
                        (NCORES, 128, GROUPS * BINS)))

    wb, wf = _pack_consts(f)
    sel = _pack_sel(f)

    # ---- z6 lookup table over the x quantization grid ----
    grid = (XLO + XSTEP * np.arange(NROWS)).astype(np.float32)
    relu = lambda a: np.maximum(a, 0.0, out=a)
    tz = relu(grid[:, None] * f["W1"][0][None, :] + f["B1"])
    tz = relu(tz @ f["W2"] + f["B2"])
    tz = relu(tz @ f["W3"] + f["B3"])
    tz = relu(tz @ f["W4"] + f["B4"])
    tz = relu(tz @ f["W5"] + f["B5"])
    tz = relu(tz @ f["W6"] + f["B6"])
    tab = tz.astype(NPBF)                       # [NROWS, 128]

    # gather indices for the tail NTG tiles of each group
    q = np.clip(np.rint((xs - XLO) / XSTEP), 0, NROWS - 1).astype(np.int64)
    # device layout: [128, GROUPS, MAXGP, 64]; idx i -> [16c + i%16, i//16]
    idxg = np.zeros((NCORES, 128, GROUPS, MAXGP, 64), np.int16)
    for g in range(GROUPS):
        NTGg = NTGS[g]
        NTPg = NT - NTGg
        qg = q[:, g, NTPg * 512:].reshape(NCORES, NTGg // 2, 1024)
        wrap = qg.astype(np.int16).reshape(NCORES, NTGg // 2, 64, 16)
        wrap = wrap.transpose(0, 3, 1, 2)       # [c, 16, K, 64]
        idxg[:, :, g, :NTGg // 2] = np.tile(wrap, (1, 8, 1, 1))

    if G not in _NC_CACHE:
        _NC_CACHE[G] = _build(G)
    nc = _NC_CACHE[G]

    in_maps = [{"xs": xsp[c], "sp": sp[c], "oh2": oh2[c], "invbc": invbc[c],
                "wbf": wb, "wfp": wf, "selc": sel, "tab": tab,
                "idxg": idxg[c]} for c in range(NCORES)]
    res = run_bass_kernel_spmd(nc, in_maps, core_ids=list(range(NCORES)))
    LAST_RESULT = res
    outs = np.stack([res.results[i]["out"] for i in range(NCORES)])
    return np.ascontiguousarray(
        outs.transpose(0, 2, 1).reshape(B, NCLS)).astype(np.float32)


# revision 42
# speedup vs baseline: 1.1706x; 1.0117x over previous
"""Trainium2 Bass kernel for AdaptedEnzymeModel (per-node MLP -> segment mean
pool -> graph MLP), SPMD over 8 NeuronCores.

Strategy (hybrid PE pipeline + table gather)
--------------------------------------------
* BN affines folded into adjacent Linears on host; device runs Linear+ReLU
  chains in bf16 (fp32 PSUM).
* Node dim sharded at graph boundaries: core c owns graphs [512c, 512c+512),
  4 groups of 128 graphs, each padded to G nodes (multiple of 2048).
* Because the node MLP has input dimension 1, z6 = MLP(x) is a function of a
  single scalar.  The host tabulates it on a 32768-level grid over x
  (step 1/2048, quantization error ~2e-4) as a [32768, 128] bf16 table.
  The last NTG tiles of each group are produced by SWDGE dma_gather
  (256B rows keyed by the quantized x) on the otherwise-idle GpSimd engine,
  bypassing the PE and both evacuation engines entirely for ~25% of nodes.
* The remaining tiles run the PE pipeline, 2-tile macros with the proven
  deferred-phase overlap:
  - L1 is an outer product: one K=64 selector matmul computes four tiles at
    once into one [128 = 4x32 feats, 512] bank (amortized over 2 macros).
  - L2: block-diagonal K=64 weights at row position 0/64 per macro parity.
  - L3/L4: 2-tile block-diagonal K=128 matmuls (one per macro).
  - L5 (K=64/tile, M=128): row-split pairs at positions (0,0)/(64,0), which
    stream concurrently on disjoint PE row bands.
  - L6 in [node, feat] layout: ones-row bias prefill + 4 K=128 block
    matmuls per tile, deferred one macro so its PE work hides evacuations.
* Segment mean-pool without one-hot streams: every 128-node block spans at
  most 2 graphs (min graph size 192), so a per-tile [128, 8] "split one-hot"
  (2 run columns per block) turns the pooling into N=2 slot matmuls that
  accumulate into a per-group [128, 8*NT] slot PSUM bank; gathered tiles
  feed the same slot matmuls.  Stage 2 (per group): evacuate slots,
  PE-transpose via identity, multiply by a [slots, bins] one-hot to get
  per-graph sums, scale by exact fp32 1/count.
* Evacuations are balanced across Scalar and Vector by phase/parity.
* Final graph MLP (128->64->32->7) on-device.
"""

import numpy as np
import ml_dtypes
from contextlib import ExitStack

import concourse.bass as bass
import concourse.tile as tile
from concourse import bacc, mybir, library_config
from concourse.bass_utils import run_bass_kernel_spmd

NCORES = 8
GROUPS = 4          # bin-groups per core
BINS = 128          # graphs per group
NCLS = 7
EPS = 1e-5
F32 = mybir.dt.float32
BF16 = mybir.dt.bfloat16
NPBF = ml_dtypes.bfloat16
RELU = mybir.ActivationFunctionType.Relu
ALU = mybir.AluOpType

LAST_RESULT = None
_NC_CACHE = {}
I16 = mybir.dt.int16
NTGS = (14, 16, 18, 20)   # gather-sourced tiles per group (later groups
                          # lean on the gather queue that outlives their PE)
MAXGP = max(NTGS) // 2
XLO, XSTEP = -8.0, 1.0 / 2048.0   # x quantization grid for the z6 table
NROWS = 32768


def _ensure_ntff_hook():
    """bass_utils' trace path needs antenv.axon_hooks, which this image's
    antenv package lacks.  Register a shim backed by trn_agent_boot's ctypes
    NTFF driver so BASS_TRACE=1 yields exec_time_ns.  Degrades silently."""
    import sys
    import types
    try:
        import antenv
        if "antenv.axon_hooks" in sys.modules:
            return
        mod = types.ModuleType("antenv.axon_hooks")
        mod._hook = None
        mod.set_axon_ntff_profile_hook = lambda h: setattr(mod, "_hook", h)
        mod.get_axon_ntff_profile_hook = lambda: mod._hook
        sys.modules["antenv.axon_hooks"] = mod
        antenv.axon_hooks = mod
        from trn_agent_boot.trn_boot import _ntff_profile_via_ctypes
        mod._hook = _ntff_profile_via_ctypes("/opt/axon/libaxon_pjrt.so")
    except Exception:
        pass


_ensure_ntff_hook()


# ---------------------------------------------------------------- host math --
def _fold(p):
    """Fold eval-mode BN affines into adjacent linears. Returns dict of f32."""
    def aff(bn):
        g, b, m, v = bn[0], bn[1], bn[2], bn[3]
        s = g / np.sqrt(v + EPS)
        return s.astype(np.float32), (b - m * s).astype(np.float32)

    s1, t1 = aff(p["ne_bn1"]); s2, t2 = aff(p["ne_bn2"])
    sc1, tc1 = aff(p["cbn1"]); sc2, tc2 = aff(p["cbn2"])
    sf1, tf1 = aff(p["fbn1"]); sf2, tf2 = aff(p["fbn2"])
    f = {}
    f["W1"] = p["ne_w1"]; f["B1"] = p["ne_b1"]
    f["W2"] = s1[:, None] * p["ne_w2"]; f["B2"] = t1 @ p["ne_w2"] + p["ne_b2"]
    f["W3"] = s2[:, None] * p["c1a_w"]; f["B3"] = t2 @ p["c1a_w"] + p["c1a_b"]
    f["W4"] = p["c1b_w"];               f["B4"] = p["c1b_b"]
    f["W5"] = sc1[:, None] * p["c2a_w"]; f["B5"] = tc1 @ p["c2a_w"] + p["c2a_b"]
    f["W6"] = p["c2b_w"];               f["B6"] = p["c2b_b"]
    f["F1"] = sc2[:, None] * p["f1_w"]; f["F1B"] = tc2 @ p["f1_w"] + p["f1_b"]
    f["F2"] = sf1[:, None] * p["f2_w"]; f["F2B"] = tf1 @ p["f2_w"] + p["f2_b"]
    f["F3"] = sf2[:, None] * p["f3_w"]; f["F3B"] = tf2 @ p["f3_w"] + p["f3_b"]
    return {k: np.asarray(v, np.float32) for k, v in f.items()}


# bf16 const block layout
def _layout_bf():
    off, c = {}, 0
    for name, ncols in [("W2D", 128),    # blockdiag(W2pad, W2pad) K=64, M=128
                        ("W3D", 128),    # blockdiag(W3, W3) K=128, M=128
                        ("W4D", 128),
                        ("W5R", 128),    # W5 replicated on rows 0:64 / 64:128
                        ("W6", 128),
                        ("ONES", 128),
                        ("B6R4", 512), ("B6BC", 512)]:
        off[name] = c
        c += ncols
    return off, c


# f32 const block: biases + final mlp + identity (for PE transpose)
def _layout_fp():
    off, c = {}, 0
    for name, ncols in [("F1", 64), ("F2", 32), ("F3", NCLS), ("IDN", 128),
                        ("B6P", 512),
                        ("B1Q", 1),   # B1 tiled x4 (4-tile packed z1)
                        ("B2S", 1), ("B3S", 1), ("B4S", 1),
                        ("B5", 1), ("F1B", 1), ("F2B", 1), ("F3B", 1)]:
        off[name] = c
        c += ncols
    return off, c


_OFFB, _CWB = _layout_bf()
_OFFF, _CWF = _layout_fp()


def _pack_consts(f):
    wb = np.zeros((128, _CWB), NPBF)

    def putb(name, arr):
        wb[:arr.shape[0], _OFFB[name]:_OFFB[name] + arr.shape[1]] = \
            arr.astype(NPBF)

    # W2D stacked twice so both (0,0) and (64,0) row positions can read it
    w2d = np.zeros((128, 128), np.float32)
    for h in (0, 64):
        w2d[h + 0:h + 32, 0:64] = f["W2"]
        w2d[h + 32:h + 64, 64:128] = f["W2"]
    putb("W2D", w2d)
    w3d = np.zeros((128, 128), np.float32)
    w3d[0:64, 0:64] = f["W3"]
    w3d[64:128, 64:128] = f["W3"]
    putb("W3D", w3d)
    w4d = np.zeros((128, 128), np.float32)
    w4d[0:64, 0:64] = f["W4"]
    w4d[64:128, 64:128] = f["W4"]
    putb("W4D", w4d)
    putb("W5R", np.tile(f["W5"], (2, 1)))
    putb("W6", f["W6"])
    wb[0, _OFFB["ONES"]:_OFFB["ONES"] + 128] = NPBF(1.0)
    wb[0, _OFFB["B6R4"]:_OFFB["B6R4"] + 512] = np.tile(f["B6"].astype(NPBF), 4)
    wb[:, _OFFB["B6BC"]:_OFFB["B6BC"] + 512] = \
        np.tile(f["B6"], 4)[None, :].astype(NPBF)

    wf = np.zeros((128, _CWF), np.float32)
    for k in ["F1", "F2", "F3"]:
        arr = f[k]
        wf[:arr.shape[0], _OFFF[k]:_OFFF[k] + arr.shape[1]] = arr
    wf[:, _OFFF["IDN"]:_OFFF["IDN"] + 128] = np.eye(128, dtype=np.float32)
    wf[:, _OFFF["B6P"]:_OFFF["B6P"] + 512] = np.tile(f["B6"], 4)[None, :]
    wf[:, _OFFF["B1Q"]] = np.tile(f["B1"], 4)
    wf[:, _OFFF["B2S"]] = np.tile(f["B2"], 2)
    wf[:, _OFFF["B3S"]] = np.tile(f["B3"], 2)
    wf[:, _OFFF["B4S"]] = np.tile(f["B4"], 2)
    wf[:128, _OFFF["B5"]] = f["B5"]
    for k, d in [("F1B", 64), ("F2B", 32), ("F3B", NCLS)]:
        wf[:d, _OFFF[k]] = f[k]
    return wb, wf


def _pack_sel(f):
    """16 selector variants for 4-tile-packed L1.  Variant v (tiles at xg rows
    4v..4v+3): [64, 128] with W1 in row 4v+j, cols 32j:32j+32."""
    sel = np.zeros((64, 16 * 128), NPBF)
    for v in range(16):
        for j in range(4):
            sel[4 * v + j, v * 128 + 32 * j: v * 128 + 32 * j + 32] = \
                f["W1"][0].astype(NPBF)
    return sel


# ------------------------------------------------------------- device build --
def _build(G):
    NT = G // 512            # 512-node tiles per group
    NXG = -(-NT // 64)       # 64-row x tiles per group
    NSLOT = 8 * NT           # slot columns per group (<= 512)
    NQ = -(-NSLOT // 128)    # stage-2 quarters
    assert G % 2048 == 0 and NSLOT <= 512

    nc = bacc.Bacc(None, target_bir_lowering=False)
    xs_d = nc.declare_dram_parameter("xs", [GROUPS, NXG, 64, 512], BF16,
                                     isOutput=False)
    sp_d = nc.declare_dram_parameter("sp", [GROUPS, NT // 4, 128, 32], BF16,
                                     isOutput=False)
    oh2_d = nc.declare_dram_parameter("oh2", [GROUPS, NQ, 128, BINS], BF16,
                                      isOutput=False)
    inv_d = nc.declare_dram_parameter("invbc", [128, GROUPS * BINS], F32,
                                      isOutput=False)
    wb_d = nc.declare_dram_parameter("wbf", [128, _CWB], BF16, isOutput=False)
    wf_d = nc.declare_dram_parameter("wfp", [128, _CWF], F32, isOutput=False)
    sel_d = nc.declare_dram_parameter("selc", [64, 16 * 128], BF16,
                                      isOutput=False)
    tab_d = nc.declare_dram_parameter("tab", [NROWS, 128], BF16,
                                      isOutput=False)
    idx_d = nc.declare_dram_parameter("idxg", [128, GROUPS, MAXGP, 64],
                                      I16, isOutput=False)
    out_d = nc.declare_dram_parameter("out", [NCLS, GROUPS * BINS], F32,
                                      isOutput=True)

    with ExitStack() as ctx:
        tc = ctx.enter_context(tile.TileContext(nc))
        cpool = ctx.enter_context(tc.tile_pool(name="const", bufs=1))
        gpool = ctx.enter_context(tc.tile_pool(name="gacc", bufs=1))
        xpool = ctx.enter_context(tc.tile_pool(name="xg", bufs=2))
        zpool = ctx.enter_context(tc.tile_pool(name="z", bufs=4))
        spool = ctx.enter_context(tc.tile_pool(name="small", bufs=8))
        psP = ctx.enter_context(tc.tile_pool(name="psP", bufs=4, space="PSUM"))
        psB = ctx.enter_context(tc.tile_pool(name="psB", bufs=3, space="PSUM"))
        psS = ctx.enter_context(tc.tile_pool(name="psS", bufs=1, space="PSUM"))

        nc.gpsimd.load_library(library_config.mlp)
        gbpool = ctx.enter_context(tc.tile_pool(name="gb", bufs=5))
        idxsb = cpool.tile([128, GROUPS, MAXGP, 64], I16)
        nc.sync.dma_start(idxsb[:], idx_d[:])
        wbsb = cpool.tile([128, _CWB], BF16)
        nc.sync.dma_start(wbsb[:], wb_d[:])
        wfsb = cpool.tile([128, _CWF], F32)
        nc.sync.dma_start(wfsb[:], wf_d[:])
        invsb = cpool.tile([128, GROUPS * BINS], F32)
        nc.sync.dma_start(invsb[:], inv_d[:])
        selsb = cpool.tile([64, 16 * 128], BF16)
        nc.sync.dma_start(selsb[:], sel_d[:])
        oh2sb = cpool.tile([128, GROUPS, NQ, BINS], BF16)
        for g in range(GROUPS):
            for q in range(NQ):
                nc.sync.dma_start(oh2sb[:, g, q, :], oh2_d[g, q])

        def WB(name, k, m):
            o = _OFFB[name]
            return wbsb[0:k, o:o + m]

        def WF(name, k, m):
            o = _OFFF[name]
            return wfsb[0:k, o:o + m]

        w2d = WB("W2D", 128, 128)
        w3d, w4d = WB("W3D", 128, 128), WB("W4D", 128, 128)
        w5r, w6 = WB("W5R", 128, 128), WB("W6", 128, 128)
        ones = WB("ONES", 128, 128)
        b6r4 = WB("B6R4", 128, 512)
        b6bc = WB("B6BC", 128, 512)
        f1, f2, f3 = WF("F1", 128, 64), WF("F2", 64, 32), WF("F3", 32, NCLS)
        idn = WF("IDN", 128, 128)
        b6p = WF("B6P", 128, 512)
        b1q, b2s = WF("B1Q", 128, 1), WF("B2S", 128, 1)
        b3s, b4s, b5 = WF("B3S", 128, 1), WF("B4S", 128, 1), WF("B5", 128, 1)
        f1b, f2b, f3b = WF("F1B", 64, 1), WF("F2B", 32, 1), WF("F3B", NCLS, 1)

        gsb = gpool.tile([128, GROUPS * BINS], F32)

        def z6phase(g, mi, z5c, spt, pslot, half):
            """L6 (ones prefill + 4 accumulating blocks) + ReLU evac; slot
            matmuls are deferred (returned as a pending item)."""
            p6 = psB.tile([128, 512], F32, tag="bg", name=f"p6_{g}_{mi}_{half}")
            nc.tensor.matmul(p6[:], ones, b6r4, start=True, stop=False,
                             skip_group_check=True)
            for s in range(4):
                nc.tensor.matmul(p6[:, s * 128:(s + 1) * 128],
                                 z5c[:, s * 128:(s + 1) * 128], w6,
                                 start=False, stop=(s == 3),
                                 skip_group_check=True)
            z6q = spool.tile([128, 512], BF16, tag="z6q",
                             name=f"z6q_{g}_{mi}_{half}")
            if (half + mi) % 2 == 0:
                nc.scalar.activation(z6q[:], p6[:], RELU)
            else:
                nc.vector.tensor_scalar(z6q[:], p6[:], 0.0, None, ALU.max)
            return (2 * mi + half, z6q, spt, pslot)

        def slotphase(item):
            t, z6q, spt, pslot = item
            u = t % 4                      # tile index within super-macro
            for s in range(4):
                sc = 8 * t + 2 * s
                spcol = 8 * u + 2 * s
                nc.tensor.matmul(pslot[:, sc:sc + 2],
                                 z6q[:, s * 128:(s + 1) * 128],
                                 spt[:, spcol:spcol + 2],
                                 start=True, stop=True,
                                 skip_group_check=True)

        def stage2(g, pslot):
            slots = spool.tile([128, 512], F32, tag="slots", name=f"slots{g}")
            nc.scalar.activation(slots[:, 0:NSLOT], pslot[:, 0:NSLOT],
                                 mybir.ActivationFunctionType.Copy)
            pT = psP.tile([128, 512], F32, tag="pk", name=f"pT{g}")
            for q in range(NQ):
                nc.tensor.transpose(pT[:, 128 * q:128 * q + 128],
                                    slots[:, 128 * q:128 * q + 128], idn)
            slotsT = spool.tile([128, 512], BF16, tag="slotsT",
                                name=f"slotsT{g}")
            nc.vector.tensor_scalar(slotsT[:, 0:128 * NQ], pT[:, 0:128 * NQ],
                                    0.0, None, ALU.add)
            pg = psP.tile([128, BINS], F32, tag="pk", name=f"pg{g}")
            for q in range(NQ):
                nc.tensor.matmul(pg[:], slotsT[:, 128 * q:128 * q + 128],
                                 oh2sb[:, g, q, :],
                                 start=(q == 0), stop=(q == NQ - 1),
                                 skip_group_check=True)
            nc.vector.tensor_tensor(gsb[:, g * BINS:(g + 1) * BINS], pg[:],
                                    invsb[:, g * BINS:(g + 1) * BINS],
                                    ALU.mult)

        def gatherphase(g, k, idxsb, pslot, spref):
            """Issue gather for tile pair (NTP+2k, NTP+2k+1) of group g."""
            gb = gbpool.tile([128, 8, 128], BF16, tag=f"gb{k % 5}",
                             name=f"gb{g}_{k}")
            nc.gpsimd.dma_gather(gb[:], tab_d[:], idxsb[:, g, k, :],
                                 1024, 1024, 128)
            return gb

        def gslotphase(g, k, gb, pslot, spg, NTP):
            for j in range(2):
                t = NTP + 2 * k + j
                u = t % 4
                for s in range(4):
                    sc = 8 * t + 2 * s
                    spcol = 8 * u + 2 * s
                    nc.tensor.matmul(pslot[:, sc:sc + 2],
                                     gb[:, 4 * j + s, :],
                                     spg[:, spcol:spcol + 2],
                                     start=True, stop=True,
                                     skip_group_check=True)

        prev = None
        pending = []
        pend_stage2 = None
        for g in range(GROUPS):
            NTG = NTGS[g]
            NTP = NT - NTG       # PE-pipeline tiles in this group
            assert NTP % 2 == 0 and NTG % 2 == 0
            xgs = {}
            for i in range(NXG):
                xg = xpool.tile([64, 512], BF16, tag=f"xg{i}",
                                name=f"xg{g}_{i}")
                nc.sync.dma_start(xg[:], xs_d[g, i])
                xgs[i] = xg

            pslot = psS.tile([128, 512], F32, tag="pslot", name=f"pslot{g}")
            z1cur = None

            # splitoh tiles covering the gather range
            spgs = {}
            for sm in range(NTP // 4, NT // 4):
                spg = spool.tile([128, 32], BF16, tag="spg",
                                 name=f"spg{g}_{sm}")
                nc.sync.dma_start(spg[:], sp_d[g, sm])
                spgs[sm] = spg
            gpend = []

            for mi in range(NTP // 2):
                # splitoh for the super-macro, loaded on its first macro
                if mi % 2 == 0:
                    spt = spool.tile([128, 32], BF16, tag="sp",
                                     name=f"sp{g}_{mi // 2}")
                    nc.sync.dma_start(spt[:], sp_d[g, mi // 2])

                    # ---- L1: one selector matmul -> 4 tiles of z1 ----
                    sm = mi // 2
                    v = sm % 16
                    p1 = psP.tile([128, 512], F32, tag="pk")
                    nc.tensor.matmul(p1[:], selsb[:, v * 128:(v + 1) * 128],
                                     xgs[(4 * sm) // 64][:],
                                     start=True, stop=True)
                    z1cur = zpool.tile([128, 512], BF16, tag="z1")
                    nc.scalar.activation(z1cur[:], p1[:], RELU, bias=b1q)
                z1h = z1cur[0:64, :] if mi % 2 == 0 else z1cur[64:128, :]

                # ---- L2: blockdiag K=64 (2 tiles) ----
                p2 = psP.tile([128, 512], F32, tag="pk")
                h = 0 if mi % 2 == 0 else 64
                nc.tensor.matmul(p2[:], w2d[h:h + 64, :], z1h,
                                 start=True, stop=True,
                                 tile_position=(h, 0))
                z2 = zpool.tile([128, 512], BF16, tag="z2")
                if mi % 2 == 0:
                    nc.scalar.activation(z2[:], p2[:], RELU, bias=b2s)
                else:
                    nc.vector.tensor_scalar(z2[:], p2[:], b2s, 0.0, ALU.add,
                                            ALU.max)

                # ---- L3: blockdiag K=128 ----
                p3 = psP.tile([128, 512], F32, tag="pk")
                nc.tensor.matmul(p3[:], w3d, z2[:], start=True, stop=True)
                z3 = zpool.tile([128, 512], BF16, tag="z3")
                nc.scalar.activation(z3[:], p3[:], RELU, bias=b3s)

                # ---- deferred L6 of the previous macro (half 0) ----
                if prev is not None:
                    g_, mi_, z5u_, z5v_, spt_, pslot_ = prev
                    pending.append(z6phase(g_, mi_, z5u_, spt_, pslot_, 0))
                    if pend_stage2 is not None:
                        stage2(*pend_stage2)
                        pend_stage2 = None

                # ---- L4 ----
                p4 = psP.tile([128, 512], F32, tag="pk")
                nc.tensor.matmul(p4[:], w4d, z3[:], start=True, stop=True)
                z4 = zpool.tile([128, 512], BF16, tag="z4")
                nc.vector.tensor_scalar(z4[:], p4[:], b4s, 0.0, ALU.add,
                                        ALU.max)

                # ---- deferred L6 of the previous macro (half 1) ----
                if prev is not None:
                    g_, mi_, z5u_, z5v_, spt_, pslot_ = prev
                    pending.append(z6phase(g_, mi_, z5v_, spt_, pslot_, 1))
                    prev = None

                # ---- drain slot matmuls two macros back ----
                while len(pending) > 2:
                    slotphase(pending.pop(0))

                # near the group end, start draining gather slots early so
                # the boundary flush never bunches PE work behind the queue
                if mi >= (NTP // 2) - 4:
                    if len(gpend) > 1:
                        k_, gb_ = gpend.pop(0)
                        gslotphase(g, k_, gb_,
                                   pslot, spgs[(NTP + 2 * k_) // 4], NTP)

                # ---- weave gather issue / gather slot matmuls ----
                step = max(1, (NTP // 2) // max(1, NTG // 2 + 1))
                if mi % step == 0:
                    k = mi // step
                    if k < NTG // 2:
                        gb = gatherphase(g, k, idxsb, pslot, None)
                        gpend.append((k, gb))
                    if len(gpend) > 3:
                        k_, gb_ = gpend.pop(0)
                        gslotphase(g, k_, gb_,
                                   pslot, spgs[(NTP + 2 * k_) // 4], NTP)

                # ---- L5: row-split pair ----
                p5u = psB.tile([128, 512], F32, tag="bg")
                nc.tensor.matmul(p5u[:], w5r[0:64, :], z4[0:64, :],
                                 start=True, stop=True, tile_position=(0, 0))
                p5v = psB.tile([128, 512], F32, tag="bg")
                nc.tensor.matmul(p5v[:], w5r[64:128, :], z4[64:128, :],
                                 start=True, stop=True, tile_position=(64, 0))
                z5u = zpool.tile([128, 512], BF16, tag="z5u")
                nc.scalar.activation(z5u[:], p5u[:], RELU, bias=b5)
                z5v = zpool.tile([128, 512], BF16, tag="z5v")
                nc.vector.tensor_scalar(z5v[:], p5v[:], b5, 0.0, ALU.add,
                                        ALU.max)

                prev = (g, mi, z5u, z5v, spt, pslot)

            for k_, gb_ in gpend:
                gslotphase(g, k_, gb_, pslot, spgs[(NTP + 2 * k_) // 4], NTP)
            gpend = []
            if prev is not None:
                g_, mi_, z5u_, z5v_, spt_, pslot_ = prev
                pending.append(z6phase(g_, mi_, z5u_, spt_, pslot_, 0))
                pending.append(z6phase(g_, mi_, z5v_, spt_, pslot_, 1))
                prev = None
            for item in pending:
                slotphase(item)
            pending = []

            pend_stage2_next = (g, pslot)
            if g == GROUPS - 1:
                # flush: last macro + stage2 of the last two groups
                if prev is not None:
                    g_, mi_, z5u_, z5v_, spt_, pslot_ = prev
                    pending.append(z6phase(g_, mi_, z5u_, spt_, pslot_, 0))
                    pending.append(z6phase(g_, mi_, z5v_, spt_, pslot_, 1))
                    prev = None
                for item in pending:
                    slotphase(item)
                pending = []
                if pend_stage2 is not None:
                    stage2(*pend_stage2)
                stage2(*pend_stage2_next)
            else:
                pend_stage2 = pend_stage2_next

        # ---- final graph MLP ----
        pf1 = psP.tile([64, 512], F32, tag="pk")
        nc.tensor.matmul(pf1[:], f1, gsb[:], start=True, stop=True)
        a1 = zpool.tile([64, 512], F32, tag="a1")
        nc.scalar.activation(a1[:], pf1[:], RELU, bias=f1b)
        pf2 = psP.tile([32, 512], F32, tag="pk")
        nc.tensor.matmul(pf2[:], f2, a1[:], start=True, stop=True)
        a2 = zpool.tile([32, 512], F32, tag="a2")
        nc.scalar.activation(a2[:], pf2[:], RELU, bias=f2b)
        pf3 = psP.tile([NCLS, 512], F32, tag="pk")
        nc.tensor.matmul(pf3[:], f3, a2[:], start=True, stop=True)
        osb = zpool.tile([NCLS, 512], F32, tag="osb")
        nc.vector.tensor_scalar(osb[:], pf3[:], f3b, None, ALU.add)
        nc.sync.dma_start(out_d[:], osb[:])

    nc.compile()
    return nc


# -------------------------------------------------------------------- entry --
def kernel(**inputs):
    global LAST_RESULT
    x = np.asarray(inputs["x"], np.float32)
    batch = np.asarray(inputs["batch"], np.int32)
    B = int(np.asarray(inputs["num_graphs"]))
    assert B == NCORES * GROUPS * BINS, f"unexpected num_graphs {B}"

    params = {k: np.asarray(v, np.float32) for k, v in inputs.items()
              if k not in ("x", "batch", "num_graphs")}
    f = _fold(params)

    bounds = np.searchsorted(batch, np.arange(0, B + 1, BINS))
    seg = bounds[1:] - bounds[:-1]
    counts = np.bincount(batch, minlength=B)
    inv = (1.0 / np.maximum(counts, 1)).astype(np.float32)
    assert counts.min() >= 128, "block-span-2 assumption violated"

    G = max(2048, int(-(-int(seg.max()) // 2048) * 2048))
    NT = G // 512
    SM = NT // 4
    NXG = -(-NT // 64)
    NSLOT = 8 * NT
    NQ = -(-NSLOT // 128)
    assert NSLOT <= 512

    xs = np.zeros((NCORES, GROUPS, G), np.float32)
    bi = np.full((NCORES, GROUPS, G), -1, np.int64)
    for c in range(NCORES):
        for g in range(GROUPS):
            k = c * GROUPS + g
            s, e = int(bounds[k]), int(bounds[k + 1])
            n = e - s
            xs[c, g, :n] = x[s:e]
            bi[c, g, :n] = (batch[s:e] - k * BINS).astype(np.int64)
    xsp = np.zeros((NCORES, GROUPS, NXG * 64, 512), np.float32)
    xsp[:, :, :NT] = xs.reshape(NCORES, GROUPS, NT, 512)
    xsp = xsp.reshape(NCORES, GROUPS, NXG, 64, 512).astype(NPBF)

    # split one-hot: block (t, s) of 128 nodes spans <= 2 graphs.
    # run 0 = first graph of the block, run 1 = second (if present).
    biB = bi.reshape(NCORES, GROUPS, NT, 4, 128)  # [c, g, t, s, node]
    first = biB[..., 0]                            # bin of node 0 (or -1)
    # a padded block ([-1...]) contributes nothing
    firstv = np.where(first < 0, 0, first)
    isfirst = (biB == firstv[..., None])
    valid = biB >= 0
    run0 = (isfirst & valid).astype(NPBF)          # [c,g,t,s,128]
    run1 = ((~isfirst) & valid).astype(NPBF)
    # sp layout: [c, g, sm, node(128), 32] with col 8*u + 2*s + r for
    # tile-in-sm u, block s, run r
    sp = np.zeros((NCORES, GROUPS, SM, 128, 32), NPBF)
    r0 = run0.transpose(0, 1, 2, 4, 3)             # [c,g,t,node,s]
    r1 = run1.transpose(0, 1, 2, 4, 3)
    for u in range(4):
        tsel = np.arange(SM) * 4 + u
        sp[:, :, :, :, 8 * u + 0:8 * u + 8:2] = r0[:, :, tsel]
        sp[:, :, :, :, 8 * u + 1:8 * u + 8:2] = r1[:, :, tsel]

    # slot -> bin map: slot 8t+2s+r of group g -> bin value
    second = np.where(valid & ~isfirst, biB, -1).max(axis=-1)  # [c,g,t,s]
    firstbin = np.where(valid.any(axis=-1), firstv, -1)
    slotbin = np.stack([firstbin, second], axis=-1)  # [c,g,t,s,2]
    slotbin = slotbin.reshape(NCORES, GROUPS, NSLOT)
    oh2 = np.zeros((NCORES, GROUPS, NQ * 128, BINS), NPBF)
    cc, gg, ss = np.nonzero(slotbin >= 0)
    oh2[cc, gg, ss, slotbin[cc, gg, ss]] = NPBF(1.0)
    oh2 = oh2.reshape(NCORES, GROUPS, NQ, 128, BINS)

    invbc = np.ascontiguousarray(
        np.broadcast_to(inv.reshape(NCORES, GROUPS * BINS)[:, None, :],
                        (NCORES, 128, GROUPS * BINS)))

    wb, wf = _pack_consts(f)
    sel = _pack_sel(f)

    # ---- z6 lookup table over the x quantization grid ----
    grid = (XLO + XSTEP * np.arange(NROWS)).astype(np.float32)
    relu = lambda a: np.maximum(a, 0.0, out=a)
    tz = relu(grid[:, None] * f["W1"][0][None, :] + f["B1"])
    tz = relu(tz @ f["W2"] + f["B2"])
    tz = relu(tz @ f["W3"] + f["B3"])
    tz = relu(tz @ f["W4"] + f["B4"])
    tz = relu(tz @ f["W5"] + f["B5"])
    tz = relu(tz @ f["W6"] + f["B6"])
    tab = tz.astype(NPBF)                       # [NROWS, 128]

    # gather indices for the tail NTG tiles of each group
    q = np.clip(np.rint((xs - XLO) / XSTEP), 0, NROWS - 1).astype(np.int64)
    # device layout: [128, GROUPS, MAXGP, 64]; idx i -> [16c + i%16, i//16]
    idxg = np.zeros((NCORES, 128, GROUPS, MAXGP, 64), np.int16)
    for g in range(GROUPS):
        NTGg = NTGS[g]
        NTPg = NT - NTGg
        qg = q[:, g, NTPg * 512:].reshape(NCORES, NTGg // 2, 1024)
        wrap = qg.astype(np.int16).reshape(NCORES, NTGg // 2, 64, 16)
        wrap = wrap.transpose(0, 3, 1, 2)       # [c, 16, K, 64]
        idxg[:, :, g, :NTGg // 2] = np.tile(wrap, (1, 8, 1, 1))

    if G not in _NC_CACHE:
        _NC_CACHE[G] = _build(G)
    nc = _NC_CACHE[G]

    in_maps = [{"xs": xsp[c], "sp": sp[c], "oh2": oh2[c], "invbc": invbc[c],
                "wbf": wb, "wfp": wf, "selc": sel, "tab": tab,
                "idxg": idxg[c]} for c in range(NCORES)]
    res = run_bass_kernel_spmd(nc, in_maps, core_ids=list(range(NCORES)))
    LAST_RESULT = res
    outs = np.stack([res.results[i]["out"] for i in range(NCORES)])
    return np.ascontiguousarray(
        outs.transpose(0, 2, 1).reshape(B, NCLS)).astype(np.float32)
